# revision 41
# baseline (speedup 1.0000x reference)
"""Trainium2 Bass kernel for nn_Decomposeable (decomposable attention).

Sharding: data-parallel over batch — 8 cores x 16 examples.
Layout strategy per core (T = 4096 tokens per side, blocks of 128 tokens,
token t = block*128 + p):
  - gather emb rows (bf16 cast in DMA), rowwise rsqrt-norm via exp(-0.5*ln(ss))
  - PE-transpose to feature-major eT [300, T]; project to x^T [200, T] (bf16)
  - PE-transpose x^T -> token-major x_tok blocks (attention values)
  - F-MLP feature-major; sim per example on PE; ACT exp with per-partition
    mask scale + accumulated row sums; the transposed exp-matrix ET comes
    from PE-transposing E1 (saves the simT matmuls and half the exps), with
    Z2 accumulated during the DVE evacuation (softmax without normalizing
    the matrix: fold 1/Z into the attention-output evacuation)
  - G layer1 feature-major (concat via K-chunks), layer2 token-major with
    bias via ones-row; relu+mask+evac fused; masked sum via indicator matmul
  - H-MLP + output head on [16, 400] (bias via ones-rows)
All matmul inputs bf16, fp32 PSUM accumulation.
Weights/consts packed into 2 dram tensors (wpack bf16 / fpack f32) loaded
with 3 large DMAs split over the sync+scalar HWDGE rings; masks precomputed
on host.
"""
import sys
import numpy as np

for _p in ('/opt/trn_rl_repo', '/root/.axon_site'):
    if _p not in sys.path:
        sys.path.insert(0, _p)

import ml_dtypes

bfloat16 = ml_dtypes.bfloat16

B, S, V, E, D, C = 128, 256, 50000, 300, 200, 3
NCORES = 8
BPC = B // NCORES          # 16 examples per core
T = BPC * S                # 4096 tokens per side per core
NB = T // 128              # 32 blocks
NGRP = NB // 4             # 8 groups (512 tokens)

# packed-weights column layout: (tag, rows, cols)
WCHUNKS = [
    ("identb", 128, 128), ("qsel", 128, 2 * BPC - 1),
    ("wp0", 128, D), ("wp1", 128, D), ("wp2", E - 256, D),
    ("fw1a", 128, D), ("fw1b", D - 128, D),
    ("fw2a", 128, D), ("fw2b", D - 128, D),
    ("gw1a", 128, D), ("gw1b", 72, D), ("gw1c", 128, D), ("gw1d", 72, D),
    ("gw2a", 128, D), ("gw2b", 73, D),
    ("hw1a", 128, D), ("hw1b", 72, D), ("hw1c", 128, D), ("hw1d", 73, D),
    ("hw2a", 128, D), ("hw2b", 73, D),
    ("woa", 128, C), ("wob", 73, C),
]
WOFF = {}
_c = 0
for _nm, _r, _ncol in WCHUNKS:
    WOFF[_nm] = (_c, _r, _ncol)
    _c += _ncol
WCOLS = _c
WSPLIT = WOFF["gw1a"][0]   # sync ring loads [0, WSPLIT), scalar ring the rest

# fpack f32 layout: mask1 [128,NB], mask2 [128,NB], then bias columns
FB_COLS = {"fb1a": 2 * NB, "fb1b": 2 * NB + 1, "fb2a": 2 * NB + 2,
           "fb2b": 2 * NB + 3, "gb1a": 2 * NB + 4, "gb1b": 2 * NB + 5}
FCOLS = 2 * NB + 6

_cache = {}


def _pin_act_table_set(bacc_mod, hw_specs):
    """Make every ACT function resolve to `natural_log_exp_and_others` so the
    kernel does exactly one ACT_TABLE_LOAD (we only use exp/ln/relu/copy)."""
    import functools
    orig = hw_specs.get_activation_tables.__wrapped__

    @functools.cache
    def pinned(arch):
        t = orig(arch)
        keep = "natural_log_exp_and_others"
        if keep not in t:
            return t
        return {name: (fns if name == keep else set())
                for name, fns in t.items()}

    bacc_mod.get_activation_tables = pinned


def _build(debug_taps=False):
    from concourse import bass, bacc, mybir, tile
    from concourse import hw_specs
    _pin_act_table_set(bacc, hw_specs)

    F32 = mybir.dt.float32
    BF16 = mybir.dt.bfloat16
    I32 = mybir.dt.int32
    AF = mybir.ActivationFunctionType
    OP = mybir.AluOpType
    X = mybir.AxisListType.X

    nc = bacc.Bacc(None, num_swdge_queues=4)

    # ---- dram I/O ----
    emb = nc.dram_tensor("emb", [V, E], F32, kind="ExternalInput")
    xi = [nc.dram_tensor(f"x{s}i", [128, NB], I32, kind="ExternalInput") for s in (1, 2)]
    wpack_in = nc.dram_tensor("wpack", [128, WCOLS], BF16, kind="ExternalInput")
    fpack_in = nc.dram_tensor("fpack", [128, FCOLS], F32, kind="ExternalInput")
    y_out = nc.dram_tensor("y", [C, BPC], F32, kind="ExternalOutput")

    taps = {}

    def tap(name, shape, dtype=F32):
        if debug_taps:
            taps[name] = nc.dram_tensor(f"tap_{name}", shape, dtype, kind="ExternalOutput")
            return taps[name]
        return None

    from concourse.tile import TileContext, add_dep_helper

    from contextlib import ExitStack
    with TileContext(nc) as tc, ExitStack() as stk:
        wp = stk.enter_context(tc.tile_pool(name="wp", bufs=1))
        sp = stk.enter_context(tc.tile_pool(name="sp", bufs=1))
        ep = stk.enter_context(tc.tile_pool(name="ep", bufs=16))
        etp = stk.enter_context(tc.tile_pool(name="etp", bufs=3))
        xp = stk.enter_context(tc.tile_pool(name="xp", bufs=3))
        fp = stk.enter_context(tc.tile_pool(name="fp", bufs=3))
        ap_ = stk.enter_context(tc.tile_pool(name="ap", bufs=3))
        gp = stk.enter_context(tc.tile_pool(name="gp", bufs=3))
        vp = stk.enter_context(tc.tile_pool(name="vp", bufs=1))
        trp = stk.enter_context(tc.tile_pool(name="trp", bufs=3, space="PSUM"))
        mmp = stk.enter_context(tc.tile_pool(name="mmp", bufs=4, space="PSUM"))

        # ---------- setup: 2 index DMAs + 3 packed DMAs (2 rings) ----------
        idx_t = [wp.tile([128, NB], I32, tag=f"idx{s}", name=f"idx{s}") for s in range(2)]
        for s in range(2):
            nc.sync.dma_start(out=idx_t[s][:], in_=xi[s][:])
        ft = wp.tile([128, FCOLS], F32, tag="ft", name="ft")
        nc.sync.dma_start(out=ft[:], in_=fpack_in[:])
        wt = wp.tile([128, WCOLS], BF16, tag="wt", name="wt")
        nc.sync.dma_start(out=wt[:, :WSPLIT], in_=wpack_in[:, :WSPLIT])
        nc.scalar.dma_start(out=wt[:, WSPLIT:], in_=wpack_in[:, WSPLIT:])

        def wv(nm):
            c0, rows, ncol = WOFF[nm]
            return wt[:rows, c0:c0 + ncol]

        identb = wv("identb")
        qsel = wv("qsel")
        wproj_k = [wv("wp0"), wv("wp1"), wv("wp2")]
        fw1_k = [wv("fw1a"), wv("fw1b")]
        fw2_k = [wv("fw2a"), wv("fw2b")]
        gw1_k = [wv("gw1a"), wv("gw1b"), wv("gw1c"), wv("gw1d")]
        gw2_k = [wv("gw2a"), wv("gw2b")]
        hw1_k = [wv("hw1a"), wv("hw1b"), wv("hw1c"), wv("hw1d")]
        hw2_k = [wv("hw2a"), wv("hw2b")]
        wout_k = [wv("woa"), wv("wob")]

        mask_tok = [ft[:, 0:NB], ft[:, NB:2 * NB]]
        fb1_t = [ft[:128, FB_COLS["fb1a"]:FB_COLS["fb1a"] + 1],
                 ft[:72, FB_COLS["fb1b"]:FB_COLS["fb1b"] + 1]]
        fb2_t = [ft[:128, FB_COLS["fb2a"]:FB_COLS["fb2a"] + 1],
                 ft[:72, FB_COLS["fb2b"]:FB_COLS["fb2b"] + 1]]
        gb1_t = [ft[:128, FB_COLS["gb1a"]:FB_COLS["gb1a"] + 1],
                 ft[:72, FB_COLS["gb1b"]:FB_COLS["gb1b"] + 1]]

        # per-side norm state
        ss_t = [sp.tile([128, NB], F32, tag=f"ss{s}", name=f"ss{s}") for s in range(2)]
        rs_t = [sp.tile([128, NB], F32, tag=f"rs{s}", name=f"rs{s}") for s in range(2)]
        ln_t = [sp.tile([128, NB], F32, tag=f"ln{s}", name=f"ln{s}") for s in range(2)]

        # v accumulators in SBUF (psum only holds one group's partial)
        v_sb = vp.tile([BPC, 2, D], F32, tag="vsb", name="vsb")
        nc.vector.memset(v_sb[:, :, :], 0.0)

        MCH = ((0, 128), (128, 200))  # feature M/K chunks of D=200

        gather_n = 0
        e_hist = []  # (tile, last_consumer_instruction) per gather, WAR deps
        EBUFS = 24

        def gather_block(s, c):
            nonlocal gather_n
            e = ep.tile([128, E], BF16, tag="e", name="e", bufs=EBUFS)
            # Gate at the measured gather cadence (~1.4us/block serialized on
            # the Pool Q7) so the list scheduler orders gather-dependent ops
            # realistically instead of queueing them ahead of ready compute.
            with tc.tile_wait_until((9000 + gather_n * 1500) / 1e6):
                g = nc.gpsimd.indirect_dma_start(
                    out=e[:], out_offset=None, in_=emb[:],
                    in_offset=bass.IndirectOffsetOnAxis(ap=idx_t[s][:, c:c + 1], axis=0))
            qn = gather_n % 4
            if qn:
                g.ins.queue = f"qPoolDynamic{qn}"
            if len(e_hist) >= EBUFS:
                prev = e_hist[len(e_hist) - EBUFS][1]
                if prev is not None:
                    add_dep_helper(g.ins, prev.ins, True, "gather WAR on recycled e slot")
            e_hist.append([e, None])
            gather_n += 1
            return e, len(e_hist) - 1

        # ---------- main loop (stage A pipelined one group ahead) ----------
        def stage_a(g):
            c0 = g * 4
            xtok = {}
            xpT = {}
            f_T = {}
            eb = {}
            eTg = {}
            # per-side front end: gather+sumsq, per-wave rsqrt, scale+transpose.
            # Side-complete ordering keeps the Vector FIFO free of cross-side
            # head-of-line blocking (side-0 scales are not queued behind
            # side-1 sumsqs that wait on serialized gathers).
            for s in range(2):
                eb[s] = []
                for c in range(c0, c0 + 4):
                    e, hidx = gather_block(s, c)
                    sq = ep.tile([128, E], BF16, tag="sq", name="sq", bufs=4)
                    nc.vector.scalar_tensor_tensor(
                        out=sq[:], in0=e[:], scalar=1.0, in1=e[:],
                        op0=OP.mult, op1=OP.mult, accum_out=ss_t[s][:, c:c + 1])
                    eb[s].append((e, hidx, c))
                eT = [etp.tile([128, 512], BF16, tag=f"eT{s}{k}", name=f"eT{s}{k}") for k in range(3)]
                eTg[s] = eT
                for wv_ in range(2):  # waves of 2 blocks
                    cw = c0 + wv_ * 2
                    nc.scalar.activation(out=ln_t[s][:, cw:cw + 2], in_=ss_t[s][:, cw:cw + 2],
                                         func=AF.Ln)
                    nc.scalar.activation(out=rs_t[s][:, cw:cw + 2], in_=ln_t[s][:, cw:cw + 2],
                                         func=AF.Exp, scale=-0.5)
                    tr = trp.tile([128, 3, 256], BF16, tag="tr", name="tr")
                    for half in range(2):
                        e, hidx, c = eb[s][wv_ * 2 + half]
                        ebf = ep.tile([128, E], BF16, tag="ebf", name="ebf", bufs=4)
                        sc = nc.vector.tensor_scalar(
                            out=ebf[:], in0=e[:], scalar1=rs_t[s][:, c:c + 1],
                            scalar2=None, op0=OP.mult)
                        e_hist[hidx][1] = sc
                        for k in range(3):
                            ksz = 128 if k < 2 else E - 256
                            nc.tensor.transpose(
                                out=tr[:ksz, k, half * 128:(half + 1) * 128],
                                in_=ebf[:, k * 128:k * 128 + ksz],
                                identity=identb[:])
                    for k in range(3):
                        ksz = 128 if k < 2 else E - 256
                        nc.vector.tensor_copy(
                            out=eT[k][:ksz, wv_ * 256:(wv_ + 1) * 256],
                            in_=tr[:ksz, k, :])
            # phase: projection (both sides interleaved)
            for s in range(2):
                xpT[s] = [xp.tile([128, 512], BF16, tag=f"xpT{s}0", name=f"xpTa{s}"),
                          xp.tile([72, 512], BF16, tag=f"xpT{s}1", name=f"xpTb{s}")]
            for mi, (m0, m1) in enumerate(MCH):
                for s in range(2):
                    ps = mmp.tile([128, 512], F32, tag="mm", name="mm")
                    for k in range(3):
                        ksz = 128 if k < 2 else E - 256
                        nc.tensor.matmul(
                            out=ps[:m1 - m0, :], lhsT=wproj_k[k][:ksz, m0:m1],
                            rhs=eTg[s][k][:ksz, :], start=(k == 0), stop=(k == 2))
                    nc.scalar.copy(out=xpT[s][mi][:, :], in_=ps[:m1 - m0, :])
            # phase: token-major x blocks
            for s in range(2):
                xtok[s] = xp.tile([128, 4, D], BF16, tag=f"xtok{s}", name=f"xtok{s}")
            for blk in range(4):
                for s in range(2):
                    tr = trp.tile([128, 3, 256], BF16, tag="tr", name="tr")
                    for mi, (m0, m1) in enumerate(MCH):
                        nc.tensor.transpose(
                            out=tr[:128, 0, m0:m1],
                            in_=xpT[s][mi][:m1 - m0, blk * 128:(blk + 1) * 128],
                            identity=identb[:m1 - m0, :m1 - m0])
                    nc.vector.tensor_copy(out=xtok[s][:, blk, :], in_=tr[:, 0, 0:D])
            # phase: F MLP layer 1 (both sides interleaved)
            fh = {s: [fp.tile([128, 512], BF16, tag=f"fh{s}0", name=f"fha{s}"),
                      fp.tile([72, 512], BF16, tag=f"fh{s}1", name=f"fhb{s}")]
                  for s in range(2)}
            for mi, (m0, m1) in enumerate(MCH):
                for s in range(2):
                    ps = mmp.tile([128, 512], F32, tag="mm", name="mm")
                    for ki, (k0, k1) in enumerate(MCH):
                        nc.tensor.matmul(
                            out=ps[:m1 - m0, :], lhsT=fw1_k[ki][:k1 - k0, m0:m1],
                            rhs=xpT[s][ki][:k1 - k0, :], start=(ki == 0), stop=(ki == 1))
                    nc.scalar.activation(out=fh[s][mi][:, :], in_=ps[:m1 - m0, :],
                                         func=AF.Relu, bias=fb1_t[mi][:])
            # phase: F MLP layer 2
            for s in range(2):
                f_T[s] = [fp.tile([128, 512], BF16, tag=f"fT{s}0", name=f"fTa{s}"),
                          fp.tile([72, 512], BF16, tag=f"fT{s}1", name=f"fTb{s}")]
            for mi, (m0, m1) in enumerate(MCH):
                for s in range(2):
                    ps = mmp.tile([128, 512], F32, tag="mm", name="mm")
                    for ki, (k0, k1) in enumerate(MCH):
                        nc.tensor.matmul(
                            out=ps[:m1 - m0, :], lhsT=fw2_k[ki][:k1 - k0, m0:m1],
                            rhs=fh[s][ki][:k1 - k0, :], start=(ki == 0), stop=(ki == 1))
                    nc.scalar.activation(out=f_T[s][mi][:, :], in_=ps[:m1 - m0, :],
                                         func=AF.Relu, bias=fb2_t[mi][:])

            if debug_taps and g == 0:
                for nm, src in (("eT0", eTg[0][0]), ("xpT0", xpT[0][0]),
                                ("fT0", f_T[0][0])):
                    t = tap(nm, [128, 512], BF16)
                    nc.sync.dma_start(out=t[:], in_=src[:])
                t = tap("xtok0", [128, 4 * D], BF16)
                nc.sync.dma_start(out=t[:], in_=xtok[0][:].rearrange("p a b -> p (a b)"))
            return xpT, xtok, f_T

        def stage_bc(g, xpT, xtok, f_T):
            c0 = g * 4
            # ---- attention + G for the 2 examples of this group ----
            # group-level betaT/alphaT (feature-major attention outputs)
            attT = {0: [ap_.tile([128, 512], BF16, tag="betaTa", name="betaTa"),
                        ap_.tile([72, 512], BF16, tag="betaTb", name="betaTb")],
                    1: [ap_.tile([128, 512], BF16, tag="alphaTa", name="alphaTa"),
                        ap_.tile([72, 512], BF16, tag="alphaTb", name="alphaTb")]}
            exd = {}
            for bl in range(2):        # phase 1: sim/simT + exp (both examples)
                b = 2 * g + bl         # global example in core
                ecol = bl * 256        # column offset of example in group tiles
                zr = ap_.tile([128, 4], F32, tag="zr", name="zr")
                rz = ap_.tile([128, 4], F32, tag="rz", name="rz")
                E1 = [ap_.tile([128, 256], BF16, tag=f"E1_{i}", name=f"E1_{i}") for i in range(2)]
                ET = [ap_.tile([128, 256], BF16, tag=f"ET_{j}", name=f"ET_{j}") for j in range(2)]
                exd[bl] = (zr, rz, E1, ET)
                for i in range(2):     # sim rows i-chunk
                    ps = mmp.tile([128, 512], F32, tag="mm", name="mm")
                    for ki in range(2):
                        kk = MCH[ki]
                        nc.tensor.matmul(
                            out=ps[:, :256],
                            lhsT=f_T[0][ki][:kk[1] - kk[0], ecol + i * 128:ecol + (i + 1) * 128],
                            rhs=f_T[1][ki][:kk[1] - kk[0], ecol:ecol + 256],
                            start=(ki == 0), stop=(ki == 1))
                    nc.scalar.activation(
                        out=E1[i][:], in_=ps[:, :256], func=AF.Exp,
                        scale=mask_tok[0][:, 2 * b + i:2 * b + i + 1],
                        accum_out=zr[:, i:i + 1])
                # ET = E1^T via PE transpose (saves the simT matmuls + exps;
                # Z2 accumulated during the DVE evacuation)
                trE = trp.tile([128, 3, 256], BF16, tag="tr", name="trE")
                for jc in range(2):
                    for i in range(2):
                        nc.tensor.transpose(
                            out=trE[:, jc, i * 128:(i + 1) * 128],
                            in_=E1[i][:, jc * 128:(jc + 1) * 128],
                            identity=identb[:])
                for jc in range(2):
                    nc.vector.tensor_scalar(
                        out=ET[jc][:], in0=trE[:, jc, :], scalar1=1.0,
                        scalar2=0.0, op0=OP.mult, op1=OP.add,
                        accum_out=zr[:, 2 + jc:3 + jc])
                nc.vector.reciprocal(out=rz[:], in_=zr[:])

            for bl in range(2):        # phase 2: attention outputs
                b = 2 * g + bl
                ecol = bl * 256
                zr, rz, E1, ET = exd[bl]
                # attention outputs, token-major, normalized at evacuation
                for kind in range(2):   # 0: beta (rows i), 1: alpha (rows j)
                    EWT = ET if kind == 0 else E1
                    vals = xtok[1] if kind == 0 else xtok[0]
                    tr = trp.tile([128, 2, 256], BF16, tag="tr", name="tr")
                    for i in range(2):
                        ps = mmp.tile([128, 512], F32, tag="mm", name="mm")
                        for jj in range(2):
                            nc.tensor.matmul(
                                out=ps[:, :D],
                                lhsT=EWT[jj][:, i * 128:(i + 1) * 128],
                                rhs=vals[:, 2 * bl + jj, :],
                                start=(jj == 0), stop=(jj == 1))
                        att_tok = ap_.tile([128, D], BF16, tag=f"att{kind}", name=f"att{kind}")
                        nc.scalar.activation(
                            out=att_tok[:], in_=ps[:, :D], func=AF.Copy,
                            scale=rz[:, 2 * kind + i:2 * kind + i + 1])
                        for mi, (m0, m1) in enumerate(MCH):
                            nc.tensor.transpose(
                                out=tr[:m1 - m0, mi, i * 128:(i + 1) * 128],
                                in_=att_tok[:, m0:m1], identity=identb[:])
                    for mi, (m0, m1) in enumerate(MCH):
                        nc.vector.tensor_copy(
                            out=attT[kind][mi][:m1 - m0, ecol:ecol + 256],
                            in_=tr[:m1 - m0, mi, :])
                if debug_taps and b == 0:
                    t = tap("E1_0", [128, 256], BF16)
                    nc.sync.dma_start(out=t[:], in_=E1[0][:])
                    t = tap("ET_0", [128, 256], BF16)
                    nc.sync.dma_start(out=t[:], in_=ET[0][:])
                    t = tap("zr0", [128, 4])
                    nc.sync.dma_start(out=t[:], in_=zr[:])
                    t = tap("attT0", [128, 512], BF16)
                    nc.sync.dma_start(out=t[:], in_=attT[0][0][:])

            # ---- G MLP per side ----
            for s in range(2):
                att = attT[0] if s == 0 else attT[1]
                hta = gp.tile([128, 512], BF16, tag="hta", name="hta")
                htb = gp.tile([73, 512], BF16, tag="htb", name="htb")
                nc.vector.memset(htb[:, :], 1.0)
                rhs_k = [xpT[s][0], xpT[s][1], att[0], att[1]]
                for mi, (m0, m1) in enumerate(MCH):
                    ps = mmp.tile([128, 512], F32, tag="mm", name="mm")
                    for ki in range(4):
                        ksz = 128 if ki % 2 == 0 else 72
                        nc.tensor.matmul(
                            out=ps[:m1 - m0, :], lhsT=gw1_k[ki][:ksz, m0:m1],
                            rhs=rhs_k[ki][:ksz, :], start=(ki == 0), stop=(ki == 3))
                    if mi == 0:
                        nc.scalar.activation(out=hta[:, :], in_=ps[:128, :],
                                             func=AF.Relu, bias=gb1_t[0][:])
                    else:
                        nc.scalar.activation(out=htb[:72, :], in_=ps[:72, :],
                                             func=AF.Relu, bias=gb1_t[1][:])
                # G2 token-major + relu + mask + v-aggregation
                ps_vg = mmp.tile([128, 512], F32, tag="mm", name="psvg")
                for blk in range(4):
                    c = c0 + blk
                    ps = mmp.tile([128, 512], F32, tag="mm", name="mm")
                    nc.tensor.matmul(out=ps[:, :D], lhsT=hta[:, blk * 128:(blk + 1) * 128],
                                     rhs=gw2_k[0][:, :], start=True, stop=False)
                    nc.tensor.matmul(out=ps[:, :D], lhsT=htb[:, blk * 128:(blk + 1) * 128],
                                     rhs=gw2_k[1][:, :], start=False, stop=True)
                    g2o = gp.tile([128, D], BF16, tag="g2o", name="g2o")
                    nc.vector.tensor_scalar(
                        out=g2o[:], in0=ps[:, :D], scalar1=0.0,
                        scalar2=mask_tok[s][:, c:c + 1],
                        op0=OP.max, op1=OP.mult)
                    bex = c // 2
                    nc.tensor.matmul(
                        out=ps_vg[:BPC, :D],
                        lhsT=qsel[:, BPC - 1 - bex:2 * BPC - 1 - bex],
                        rhs=g2o[:, :], start=(blk == 0), stop=(blk == 3))
                    if debug_taps and c == 0 and s == 0:
                        t = tap("g2o0", [128, D])
                        if t is not None:
                            g2f = sp.tile([128, D], F32, tag="g2f", name="g2f")
                            nc.vector.tensor_copy(out=g2f[:], in_=g2o[:])
                            nc.sync.dma_start(out=t[:], in_=g2f[:])
                # fold this group's partial into the SBUF v accumulator
                nc.vector.tensor_tensor(
                    out=v_sb[:, s, :], in0=v_sb[:, s, :], in1=ps_vg[:BPC, :D],
                    op=OP.add)

        pend = stage_a(0)
        for g in range(NGRP):
            stage_bc(g, *pend)
            pend = stage_a(g + 1) if g + 1 < NGRP else None

        # ---------- H MLP tail ----------
        Vb = [vp.tile([BPC, D], BF16, tag=f"V{s}", name=f"V{s}") for s in range(2)]
        for s in range(2):
            nc.vector.tensor_copy(out=Vb[s][:], in_=v_sb[:, s, :])
        if debug_taps:
            t = tap("V0", [BPC, D])
            if t is not None:
                nc.sync.dma_start(out=t[:], in_=v_sb[:, 0, :])
        # transpose V -> feature-major vT chunks
        vT = []  # 4 chunks: v1a[128,16] v1b[72,16] v2a[128,16] v2b[73,16(+ones)]
        for s in range(2):
            tr = trp.tile([128, 3, 256], BF16, tag="tr", name="tr")
            for mi, (m0, m1) in enumerate(MCH):
                nc.tensor.transpose(out=tr[:m1 - m0, mi, :BPC],
                                    in_=Vb[s][:, m0:m1], identity=identb[:BPC, :BPC])
            va = vp.tile([128, BPC], BF16, tag=f"vTa{s}", name=f"vTa{s}")
            nc.vector.tensor_copy(out=va[:], in_=tr[:128, 0, :BPC])
            szb = 73 if s == 1 else 72
            vb = vp.tile([szb, BPC], BF16, tag=f"vTb{s}", name=f"vTb{s}")
            if s == 1:
                nc.vector.memset(vb[:, :], 1.0)
            nc.vector.tensor_copy(out=vb[:72, :], in_=tr[:72, 1, :BPC])
            vT += [va, vb]

        h1a = vp.tile([128, BPC], BF16, tag="h1a", name="h1a")
        h1b = vp.tile([73, BPC], BF16, tag="h1b", name="h1b")
        nc.vector.memset(h1b[:, :], 1.0)
        for mi, (m0, m1) in enumerate(MCH):
            ps = mmp.tile([128, 512], F32, tag="mm", name="mm")
            for ki in range(4):
                ksz = [128, 72, 128, 73][ki]
                nc.tensor.matmul(out=ps[:m1 - m0, :BPC], lhsT=hw1_k[ki][:ksz, m0:m1],
                                 rhs=vT[ki][:ksz, :], start=(ki == 0), stop=(ki == 3))
            if mi == 0:
                nc.scalar.activation(out=h1a[:, :], in_=ps[:128, :BPC], func=AF.Relu)
            else:
                nc.scalar.activation(out=h1b[:72, :], in_=ps[:72, :BPC], func=AF.Relu)
        h2a = vp.tile([128, BPC], BF16, tag="h2a", name="h2a")
        h2b = vp.tile([73, BPC], BF16, tag="h2b", name="h2b")
        nc.vector.memset(h2b[:, :], 1.0)
        for mi, (m0, m1) in enumerate(MCH):
            ps = mmp.tile([128, 512], F32, tag="mm", name="mm")
            for ki in range(2):
                ksz = [128, 73][ki]
                nc.tensor.matmul(out=ps[:m1 - m0, :BPC], lhsT=hw2_k[ki][:ksz, m0:m1],
                                 rhs=[h1a, h1b][ki][:ksz, :], start=(ki == 0), stop=(ki == 1))
            if mi == 0:
                nc.scalar.activation(out=h2a[:, :], in_=ps[:128, :BPC], func=AF.Relu)
            else:
                nc.scalar.activation(out=h2b[:72, :], in_=ps[:72, :BPC], func=AF.Relu)
        ps = mmp.tile([128, 512], F32, tag="mm", name="mm")
        for ki in range(2):
            ksz = [128, 73][ki]
            nc.tensor.matmul(out=ps[:C, :BPC], lhsT=wout_k[ki][:ksz, :],
                             rhs=[h2a, h2b][ki][:ksz, :], start=(ki == 0), stop=(ki == 1))
        y_sb = vp.tile([C, BPC], F32, tag="ysb", name="ysb")
        nc.vector.tensor_copy(out=y_sb[:], in_=ps[:C, :BPC])
        nc.sync.dma_start(out=y_out[:], in_=y_sb[:])

    nc.finalize()
    return nc, taps


def _host_prep(inputs):
    """Build per-core input maps from full inputs."""
    x1 = np.asarray(inputs['x1'], dtype=np.int32)
    x2 = np.asarray(inputs['x2'], dtype=np.int32)
    len1 = np.asarray(inputs['len1'], dtype=np.int64)
    len2 = np.asarray(inputs['len2'], dtype=np.int64)
    emb = np.ascontiguousarray(np.asarray(inputs['emb'], dtype=np.float32))

    def bf(x):
        return np.asarray(x, dtype=np.float32).astype(bfloat16)

    gw2e = np.vstack([np.asarray(inputs['G_W2'], np.float32),
                      np.asarray(inputs['G_b2'], np.float32)[None, :]])
    hw1e = np.vstack([np.asarray(inputs['H_W1'], np.float32),
                      np.asarray(inputs['H_b1'], np.float32)[None, :]])
    hw2e = np.vstack([np.asarray(inputs['H_W2'], np.float32),
                      np.asarray(inputs['H_b2'], np.float32)[None, :]])
    woute = np.vstack([np.asarray(inputs['W_out'], np.float32),
                       np.asarray(inputs['b_out'], np.float32)[None, :]])
    qsel = np.zeros((128, 2 * BPC - 1), dtype=np.float32)
    qsel[:, BPC - 1] = 1.0

    wsrc = {
        "identb": np.eye(128, dtype=np.float32),
        "qsel": qsel,
        "wp0": np.asarray(inputs['W_proj'], np.float32)[0:128],
        "wp1": np.asarray(inputs['W_proj'], np.float32)[128:256],
        "wp2": np.asarray(inputs['W_proj'], np.float32)[256:E],
        "fw1a": np.asarray(inputs['F_W1'], np.float32)[0:128],
        "fw1b": np.asarray(inputs['F_W1'], np.float32)[128:D],
        "fw2a": np.asarray(inputs['F_W2'], np.float32)[0:128],
        "fw2b": np.asarray(inputs['F_W2'], np.float32)[128:D],
        "gw1a": np.asarray(inputs['G_W1'], np.float32)[0:128],
        "gw1b": np.asarray(inputs['G_W1'], np.float32)[128:200],
        "gw1c": np.asarray(inputs['G_W1'], np.float32)[200:328],
        "gw1d": np.asarray(inputs['G_W1'], np.float32)[328:400],
        "gw2a": gw2e[0:128], "gw2b": gw2e[128:201],
        "hw1a": hw1e[0:128], "hw1b": hw1e[128:200],
        "hw1c": hw1e[200:328], "hw1d": hw1e[328:401],
        "hw2a": hw2e[0:128], "hw2b": hw2e[128:201],
        "woa": woute[0:128], "wob": woute[128:201],
    }
    wpack = np.zeros((128, WCOLS), dtype=bfloat16)
    for nm, (c0, rows, ncol) in WOFF.items():
        wpack[:rows, c0:c0 + ncol] = bf(wsrc[nm])
    wpack = np.ascontiguousarray(wpack)

    fb1 = np.asarray(inputs['F_b1'], np.float32)
    fb2 = np.asarray(inputs['F_b2'], np.float32)
    gb1 = np.asarray(inputs['G_b1'], np.float32)

    iota = np.arange(128, dtype=np.float32)

    in_maps = []
    for core in range(NCORES):
        sl = slice(core * BPC, (core + 1) * BPC)

        def idx_of(x):
            return np.ascontiguousarray(
                x[sl].reshape(-1).reshape(NB, 128).T.astype(np.int32))

        def mask_of(ln):
            c = np.arange(NB)
            thr = ln[sl][c // 2] - 128.0 * (c % 2)
            return (iota[:, None] < thr[None, :]).astype(np.float32)

        fpack = np.zeros((128, FCOLS), dtype=np.float32)
        fpack[:, 0:NB] = mask_of(len1)
        fpack[:, NB:2 * NB] = mask_of(len2)
        fpack[0:128, FB_COLS["fb1a"]] = fb1[0:128]
        fpack[0:72, FB_COLS["fb1b"]] = fb1[128:D]
        fpack[0:128, FB_COLS["fb2a"]] = fb2[0:128]
        fpack[0:72, FB_COLS["fb2b"]] = fb2[128:D]
        fpack[0:128, FB_COLS["gb1a"]] = gb1[0:128]
        fpack[0:72, FB_COLS["gb1b"]] = gb1[128:D]

        in_maps.append(dict(
            emb=emb, x1i=idx_of(x1), x2i=idx_of(x2),
            wpack=wpack, fpack=np.ascontiguousarray(fpack),
        ))
    return in_maps


def run(inputs, debug_taps=False, trace=False):
    key = debug_taps
    if key not in _cache:
        _cache[key] = _build(debug_taps)
    nc, taps = _cache[key]
    in_maps = _host_prep(inputs)
    from concourse.bass_utils import run_bass_kernel_spmd
    res = run_bass_kernel_spmd(nc, in_maps, list(range(NCORES)), trace=trace)
    y = np.concatenate([r['y'].T for r in res.results], axis=0)
    return y.astype(np.float32), res


def kernel(**inputs) -> np.ndarray:
    y, _ = run(inputs)
    return y


# revision 42
# speedup vs baseline: 1.0248x; 1.0248x over previous
"""Trainium2 Bass kernel for nn_Decomposeable (decomposable attention).

Sharding: data-parallel over batch — 8 cores x 16 examples.
Layout strategy per core (T = 4096 tokens per side, blocks of 128 tokens,
token t = block*128 + p):
  - gather emb rows (bf16 cast in DMA), rowwise rsqrt-norm via exp(-0.5*ln(ss))
  - PE-transpose to feature-major eT [300, T]; project to x^T [200, T] (bf16)
  - PE-transpose x^T -> token-major x_tok blocks (attention values)
  - F-MLP feature-major; sim per example on PE; ACT exp with per-partition
    mask scale + accumulated row sums; the transposed exp-matrix ET comes
    from PE-transposing E1 (saves the simT matmuls and half the exps), with
    Z2 accumulated during the DVE evacuation (softmax without normalizing
    the matrix: fold 1/Z into the attention-output evacuation)
  - G layer1 feature-major (concat via K-chunks), layer2 token-major with
    bias via ones-row; relu+mask+evac fused; masked sum via indicator matmul
  - H-MLP + output head on [16, 400] (bias via ones-rows)
All matmul inputs bf16, fp32 PSUM accumulation.
Weights/consts packed into 2 dram tensors (wpack bf16 / fpack f32) loaded
with 3 large DMAs split over the sync+scalar HWDGE rings; masks precomputed
on host.
"""
import sys
import numpy as np

for _p in ('/opt/trn_rl_repo', '/root/.axon_site'):
    if _p not in sys.path:
        sys.path.insert(0, _p)

import ml_dtypes

bfloat16 = ml_dtypes.bfloat16

B, S, V, E, D, C = 128, 256, 50000, 300, 200, 3
NCORES = 8
BPC = B // NCORES          # 16 examples per core
T = BPC * S                # 4096 tokens per side per core
NB = T // 128              # 32 blocks
NGRP = NB // 4             # 8 groups (512 tokens)

# packed-weights column layout: (tag, rows, cols)
WCHUNKS = [
    ("identb", 128, 128), ("qsel", 128, 2 * BPC - 1),
    ("wp0", 128, D), ("wp1", 128, D), ("wp2", E - 256, D),
    ("fw1a", 128, D), ("fw1b", D - 128, D),
    ("fw2a", 128, D), ("fw2b", D - 128, D),
    ("gw1a", 128, D), ("gw1b", 72, D), ("gw1c", 128, D), ("gw1d", 72, D),
    ("gw2a", 128, D), ("gw2b", 73, D),
    ("hw1a", 128, D), ("hw1b", 72, D), ("hw1c", 128, D), ("hw1d", 73, D),
    ("hw2a", 128, D), ("hw2b", 73, D),
    ("woa", 128, C), ("wob", 73, C),
]
WOFF = {}
_c = 0
for _nm, _r, _ncol in WCHUNKS:
    WOFF[_nm] = (_c, _r, _ncol)
    _c += _ncol
WCOLS = _c
WSPLIT = WOFF["gw1a"][0]   # sync ring loads [0, WSPLIT), scalar ring the rest

# fpack f32 layout: mask1 [128,NB], mask2 [128,NB], then bias columns
FB_COLS = {"fb1a": 2 * NB, "fb1b": 2 * NB + 1, "fb2a": 2 * NB + 2,
           "fb2b": 2 * NB + 3, "gb1a": 2 * NB + 4, "gb1b": 2 * NB + 5}
FCOLS = 2 * NB + 6

_cache = {}


def _pin_act_table_set(bacc_mod, hw_specs):
    """Make every ACT function resolve to `natural_log_exp_and_others` so the
    kernel does exactly one ACT_TABLE_LOAD (we only use exp/ln/relu/copy)."""
    import functools
    orig = hw_specs.get_activation_tables.__wrapped__

    @functools.cache
    def pinned(arch):
        t = orig(arch)
        keep = "natural_log_exp_and_others"
        if keep not in t:
            return t
        return {name: (fns if name == keep else set())
                for name, fns in t.items()}

    bacc_mod.get_activation_tables = pinned


def _build(debug_taps=False):
    from concourse import bass, bacc, mybir, tile
    from concourse import hw_specs
    _pin_act_table_set(bacc, hw_specs)

    F32 = mybir.dt.float32
    BF16 = mybir.dt.bfloat16
    I32 = mybir.dt.int32
    AF = mybir.ActivationFunctionType
    OP = mybir.AluOpType
    X = mybir.AxisListType.X

    nc = bacc.Bacc(None, num_swdge_queues=4)

    # ---- dram I/O ----
    emb = nc.dram_tensor("emb", [V, E], F32, kind="ExternalInput")
    xi = [nc.dram_tensor(f"x{s}i", [128, NB], I32, kind="ExternalInput") for s in (1, 2)]
    wpack_in = nc.dram_tensor("wpack", [128, WCOLS], BF16, kind="ExternalInput")
    fpack_in = nc.dram_tensor("fpack", [128, FCOLS], F32, kind="ExternalInput")
    y_out = nc.dram_tensor("y", [C, BPC], F32, kind="ExternalOutput")

    taps = {}

    def tap(name, shape, dtype=F32):
        if debug_taps:
            taps[name] = nc.dram_tensor(f"tap_{name}", shape, dtype, kind="ExternalOutput")
            return taps[name]
        return None

    from concourse.tile import TileContext, add_dep_helper

    from contextlib import ExitStack
    with TileContext(nc) as tc, ExitStack() as stk:
        wp = stk.enter_context(tc.tile_pool(name="wp", bufs=1))
        sp = stk.enter_context(tc.tile_pool(name="sp", bufs=1))
        ep = stk.enter_context(tc.tile_pool(name="ep", bufs=16))
        etp = stk.enter_context(tc.tile_pool(name="etp", bufs=3))
        xp = stk.enter_context(tc.tile_pool(name="xp", bufs=3))
        fp = stk.enter_context(tc.tile_pool(name="fp", bufs=3))
        ap_ = stk.enter_context(tc.tile_pool(name="ap", bufs=3))
        gp = stk.enter_context(tc.tile_pool(name="gp", bufs=3))
        vp = stk.enter_context(tc.tile_pool(name="vp", bufs=1))
        trp = stk.enter_context(tc.tile_pool(name="trp", bufs=3, space="PSUM"))
        mmp = stk.enter_context(tc.tile_pool(name="mmp", bufs=4, space="PSUM"))

        # ---------- setup: 2 index DMAs + 3 packed DMAs (2 rings) ----------
        idx_t = [wp.tile([128, NB], I32, tag=f"idx{s}", name=f"idx{s}") for s in range(2)]
        for s in range(2):
            nc.sync.dma_start(out=idx_t[s][:], in_=xi[s][:])
        ft = wp.tile([128, FCOLS], F32, tag="ft", name="ft")
        nc.sync.dma_start(out=ft[:], in_=fpack_in[:])
        wt = wp.tile([128, WCOLS], BF16, tag="wt", name="wt")
        nc.sync.dma_start(out=wt[:, :WSPLIT], in_=wpack_in[:, :WSPLIT])
        nc.scalar.dma_start(out=wt[:, WSPLIT:], in_=wpack_in[:, WSPLIT:])

        def wv(nm):
            c0, rows, ncol = WOFF[nm]
            return wt[:rows, c0:c0 + ncol]

        identb = wv("identb")
        qsel = wv("qsel")
        wproj_k = [wv("wp0"), wv("wp1"), wv("wp2")]
        fw1_k = [wv("fw1a"), wv("fw1b")]
        fw2_k = [wv("fw2a"), wv("fw2b")]
        gw1_k = [wv("gw1a"), wv("gw1b"), wv("gw1c"), wv("gw1d")]
        gw2_k = [wv("gw2a"), wv("gw2b")]
        hw1_k = [wv("hw1a"), wv("hw1b"), wv("hw1c"), wv("hw1d")]
        hw2_k = [wv("hw2a"), wv("hw2b")]
        wout_k = [wv("woa"), wv("wob")]

        mask_tok = [ft[:, 0:NB], ft[:, NB:2 * NB]]
        fb1_t = [ft[:128, FB_COLS["fb1a"]:FB_COLS["fb1a"] + 1],
                 ft[:72, FB_COLS["fb1b"]:FB_COLS["fb1b"] + 1]]
        fb2_t = [ft[:128, FB_COLS["fb2a"]:FB_COLS["fb2a"] + 1],
                 ft[:72, FB_COLS["fb2b"]:FB_COLS["fb2b"] + 1]]
        gb1_t = [ft[:128, FB_COLS["gb1a"]:FB_COLS["gb1a"] + 1],
                 ft[:72, FB_COLS["gb1b"]:FB_COLS["gb1b"] + 1]]

        # per-side norm state
        ss_t = [sp.tile([128, NB], F32, tag=f"ss{s}", name=f"ss{s}") for s in range(2)]
        rs_t = [sp.tile([128, NB], F32, tag=f"rs{s}", name=f"rs{s}") for s in range(2)]
        ln_t = [sp.tile([128, NB], F32, tag=f"ln{s}", name=f"ln{s}") for s in range(2)]

        # v accumulators in SBUF (psum only holds one group's partial)
        v_sb = vp.tile([BPC, 2, D], F32, tag="vsb", name="vsb")
        nc.vector.memset(v_sb[:, :, :], 0.0)

        MCH = ((0, 128), (128, 200))  # feature M/K chunks of D=200

        gather_n = 0
        e_hist = []  # (tile, last_consumer_instruction) per gather, WAR deps
        EBUFS = 24

        def gather_block(s, c):
            nonlocal gather_n
            e = ep.tile([128, E], BF16, tag="e", name="e", bufs=EBUFS)
            # Gate at the measured gather cadence (~1.4us/block serialized on
            # the Pool Q7) so the list scheduler orders gather-dependent ops
            # realistically instead of queueing them ahead of ready compute.
            with tc.tile_wait_until((9000 + gather_n * 1400) / 1e6):
                g = nc.gpsimd.indirect_dma_start(
                    out=e[:], out_offset=None, in_=emb[:],
                    in_offset=bass.IndirectOffsetOnAxis(ap=idx_t[s][:, c:c + 1], axis=0))
            qn = gather_n % 4
            if qn:
                g.ins.queue = f"qPoolDynamic{qn}"
            if len(e_hist) >= EBUFS:
                prev = e_hist[len(e_hist) - EBUFS][1]
                if prev is not None:
                    add_dep_helper(g.ins, prev.ins, True, "gather WAR on recycled e slot")
            e_hist.append([e, None])
            gather_n += 1
            return e, len(e_hist) - 1

        # ---------- main loop (stage A pipelined one group ahead) ----------
        def stage_a(g):
            c0 = g * 4
            xtok = {}
            xpT = {}
            f_T = {}
            eb = {}
            eTg = {}
            # per-side front end: gather+sumsq, per-wave rsqrt, scale+transpose.
            # Side-complete ordering keeps the Vector FIFO free of cross-side
            # head-of-line blocking (side-0 scales are not queued behind
            # side-1 sumsqs that wait on serialized gathers).
            for s in range(2):
                eb[s] = []
                for c in range(c0, c0 + 4):
                    e, hidx = gather_block(s, c)
                    sq = ep.tile([128, E], BF16, tag="sq", name="sq", bufs=4)
                    nc.vector.scalar_tensor_tensor(
                        out=sq[:], in0=e[:], scalar=1.0, in1=e[:],
                        op0=OP.mult, op1=OP.mult, accum_out=ss_t[s][:, c:c + 1])
                    eb[s].append((e, hidx, c))
                eT = [etp.tile([128, 512], BF16, tag=f"eT{s}{k}", name=f"eT{s}{k}") for k in range(3)]
                eTg[s] = eT
                for wv_ in range(2):  # waves of 2 blocks
                    cw = c0 + wv_ * 2
                    nc.scalar.activation(out=ln_t[s][:, cw:cw + 2], in_=ss_t[s][:, cw:cw + 2],
                                         func=AF.Ln)
                    nc.scalar.activation(out=rs_t[s][:, cw:cw + 2], in_=ln_t[s][:, cw:cw + 2],
                                         func=AF.Exp, scale=-0.5)
                    tr = trp.tile([128, 3, 256], BF16, tag="tr", name="tr")
                    for half in range(2):
                        e, hidx, c = eb[s][wv_ * 2 + half]
                        ebf = ep.tile([128, E], BF16, tag="ebf", name="ebf", bufs=4)
                        sc = nc.vector.tensor_scalar(
                            out=ebf[:], in0=e[:], scalar1=rs_t[s][:, c:c + 1],
                            scalar2=None, op0=OP.mult)
                        e_hist[hidx][1] = sc
                        for k in range(3):
                            ksz = 128 if k < 2 else E - 256
                            nc.tensor.transpose(
                                out=tr[:ksz, k, half * 128:(half + 1) * 128],
                                in_=ebf[:, k * 128:k * 128 + ksz],
                                identity=identb[:])
                    for k in range(3):
                        ksz = 128 if k < 2 else E - 256
                        nc.vector.tensor_copy(
                            out=eT[k][:ksz, wv_ * 256:(wv_ + 1) * 256],
                            in_=tr[:ksz, k, :])
            # phase: projection (both sides interleaved)
            for s in range(2):
                xpT[s] = [xp.tile([128, 512], BF16, tag=f"xpT{s}0", name=f"xpTa{s}"),
                          xp.tile([72, 512], BF16, tag=f"xpT{s}1", name=f"xpTb{s}")]
            for mi, (m0, m1) in enumerate(MCH):
                for s in range(2):
                    ps = mmp.tile([128, 512], F32, tag="mm", name="mm")
                    for k in range(3):
                        ksz = 128 if k < 2 else E - 256
                        nc.tensor.matmul(
                            out=ps[:m1 - m0, :], lhsT=wproj_k[k][:ksz, m0:m1],
                            rhs=eTg[s][k][:ksz, :], start=(k == 0), stop=(k == 2))
                    nc.scalar.copy(out=xpT[s][mi][:, :], in_=ps[:m1 - m0, :])
            # phase: token-major x blocks
            for s in range(2):
                xtok[s] = xp.tile([128, 4, D], BF16, tag=f"xtok{s}", name=f"xtok{s}")
            for blk in range(4):
                for s in range(2):
                    tr = trp.tile([128, 3, 256], BF16, tag="tr", name="tr")
                    for mi, (m0, m1) in enumerate(MCH):
                        nc.tensor.transpose(
                            out=tr[:128, 0, m0:m1],
                            in_=xpT[s][mi][:m1 - m0, blk * 128:(blk + 1) * 128],
                            identity=identb[:m1 - m0, :m1 - m0])
                    nc.vector.tensor_copy(out=xtok[s][:, blk, :], in_=tr[:, 0, 0:D])
            # phase: F MLP layer 1 (both sides interleaved)
            fh = {s: [fp.tile([128, 512], BF16, tag=f"fh{s}0", name=f"fha{s}"),
                      fp.tile([72, 512], BF16, tag=f"fh{s}1", name=f"fhb{s}")]
                  for s in range(2)}
            for mi, (m0, m1) in enumerate(MCH):
                for s in range(2):
                    ps = mmp.tile([128, 512], F32, tag="mm", name="mm")
                    for ki, (k0, k1) in enumerate(MCH):
                        nc.tensor.matmul(
                            out=ps[:m1 - m0, :], lhsT=fw1_k[ki][:k1 - k0, m0:m1],
                            rhs=xpT[s][ki][:k1 - k0, :], start=(ki == 0), stop=(ki == 1))
                    nc.scalar.activation(out=fh[s][mi][:, :], in_=ps[:m1 - m0, :],
                                         func=AF.Relu, bias=fb1_t[mi][:])
            # phase: F MLP layer 2
            for s in range(2):
                f_T[s] = [fp.tile([128, 512], BF16, tag=f"fT{s}0", name=f"fTa{s}"),
                          fp.tile([72, 512], BF16, tag=f"fT{s}1", name=f"fTb{s}")]
            for mi, (m0, m1) in enumerate(MCH):
                for s in range(2):
                    ps = mmp.tile([128, 512], F32, tag="mm", name="mm")
                    for ki, (k0, k1) in enumerate(MCH):
                        nc.tensor.matmul(
                            out=ps[:m1 - m0, :], lhsT=fw2_k[ki][:k1 - k0, m0:m1],
                            rhs=fh[s][ki][:k1 - k0, :], start=(ki == 0), stop=(ki == 1))
                    nc.scalar.activation(out=f_T[s][mi][:, :], in_=ps[:m1 - m0, :],
                                         func=AF.Relu, bias=fb2_t[mi][:])

            if debug_taps and g == 0:
                for nm, src in (("eT0", eTg[0][0]), ("xpT0", xpT[0][0]),
                                ("fT0", f_T[0][0])):
                    t = tap(nm, [128, 512], BF16)
                    nc.sync.dma_start(out=t[:], in_=src[:])
                t = tap("xtok0", [128, 4 * D], BF16)
                nc.sync.dma_start(out=t[:], in_=xtok[0][:].rearrange("p a b -> p (a b)"))
            return xpT, xtok, f_T

        def stage_bc(g, xpT, xtok, f_T):
            c0 = g * 4
            # ---- attention + G for the 2 examples of this group ----
            # group-level betaT/alphaT (feature-major attention outputs)
            attT = {0: [ap_.tile([128, 512], BF16, tag="betaTa", name="betaTa"),
                        ap_.tile([72, 512], BF16, tag="betaTb", name="betaTb")],
                    1: [ap_.tile([128, 512], BF16, tag="alphaTa", name="alphaTa"),
                        ap_.tile([72, 512], BF16, tag="alphaTb", name="alphaTb")]}
            exd = {}
            for bl in range(2):        # phase 1: sim/simT + exp (both examples)
                b = 2 * g + bl         # global example in core
                ecol = bl * 256        # column offset of example in group tiles
                zr = ap_.tile([128, 4], F32, tag="zr", name="zr")
                rz = ap_.tile([128, 4], F32, tag="rz", name="rz")
                E1 = [ap_.tile([128, 256], BF16, tag=f"E1_{i}", name=f"E1_{i}") for i in range(2)]
                ET = [ap_.tile([128, 256], BF16, tag=f"ET_{j}", name=f"ET_{j}") for j in range(2)]
                exd[bl] = (zr, rz, E1, ET)
                for i in range(2):     # sim rows i-chunk
                    ps = mmp.tile([128, 512], F32, tag="mm", name="mm")
                    for ki in range(2):
                        kk = MCH[ki]
                        nc.tensor.matmul(
                            out=ps[:, :256],
                            lhsT=f_T[0][ki][:kk[1] - kk[0], ecol + i * 128:ecol + (i + 1) * 128],
                            rhs=f_T[1][ki][:kk[1] - kk[0], ecol:ecol + 256],
                            start=(ki == 0), stop=(ki == 1))
                    nc.scalar.activation(
                        out=E1[i][:], in_=ps[:, :256], func=AF.Exp,
                        scale=mask_tok[0][:, 2 * b + i:2 * b + i + 1],
                        accum_out=zr[:, i:i + 1])
                # ET = E1^T via PE transpose (saves the simT matmuls + exps;
                # Z2 accumulated during the DVE evacuation)
                trE = trp.tile([128, 3, 256], BF16, tag="tr", name="trE")
                for jc in range(2):
                    for i in range(2):
                        nc.tensor.transpose(
                            out=trE[:, jc, i * 128:(i + 1) * 128],
                            in_=E1[i][:, jc * 128:(jc + 1) * 128],
                            identity=identb[:])
                for jc in range(2):
                    nc.vector.tensor_scalar(
                        out=ET[jc][:], in0=trE[:, jc, :], scalar1=1.0,
                        scalar2=0.0, op0=OP.mult, op1=OP.add,
                        accum_out=zr[:, 2 + jc:3 + jc])
                nc.vector.reciprocal(out=rz[:], in_=zr[:])

            for bl in range(2):        # phase 2: attention outputs
                b = 2 * g + bl
                ecol = bl * 256
                zr, rz, E1, ET = exd[bl]
                # attention outputs, token-major, normalized at evacuation
                for kind in range(2):   # 0: beta (rows i), 1: alpha (rows j)
                    EWT = ET if kind == 0 else E1
                    vals = xtok[1] if kind == 0 else xtok[0]
                    tr = trp.tile([128, 2, 256], BF16, tag="tr", name="tr")
                    for i in range(2):
                        ps = mmp.tile([128, 512], F32, tag="mm", name="mm")
                        for jj in range(2):
                            nc.tensor.matmul(
                                out=ps[:, :D],
                                lhsT=EWT[jj][:, i * 128:(i + 1) * 128],
                                rhs=vals[:, 2 * bl + jj, :],
                                start=(jj == 0), stop=(jj == 1))
                        att_tok = ap_.tile([128, D], BF16, tag=f"att{kind}", name=f"att{kind}")
                        nc.scalar.activation(
                            out=att_tok[:], in_=ps[:, :D], func=AF.Copy,
                            scale=rz[:, 2 * kind + i:2 * kind + i + 1])
                        for mi, (m0, m1) in enumerate(MCH):
                            nc.tensor.transpose(
                                out=tr[:m1 - m0, mi, i * 128:(i + 1) * 128],
                                in_=att_tok[:, m0:m1], identity=identb[:])
                    for mi, (m0, m1) in enumerate(MCH):
                        nc.vector.tensor_copy(
                            out=attT[kind][mi][:m1 - m0, ecol:ecol + 256],
                            in_=tr[:m1 - m0, mi, :])
                if debug_taps and b == 0:
                    t = tap("E1_0", [128, 256], BF16)
                    nc.sync.dma_start(out=t[:], in_=E1[0][:])
                    t = tap("ET_0", [128, 256], BF16)
                    nc.sync.dma_start(out=t[:], in_=ET[0][:])
                    t = tap("zr0", [128, 4])
                    nc.sync.dma_start(out=t[:], in_=zr[:])
                    t = tap("attT0", [128, 512], BF16)
                    nc.sync.dma_start(out=t[:], in_=attT[0][0][:])

            # ---- G MLP per side ----
            for s in range(2):
                att = attT[0] if s == 0 else attT[1]
                hta = gp.tile([128, 512], BF16, tag="hta", name="hta")
                htb = gp.tile([73, 512], BF16, tag="htb", name="htb")
                nc.vector.memset(htb[:, :], 1.0)
                rhs_k = [xpT[s][0], xpT[s][1], att[0], att[1]]
                for mi, (m0, m1) in enumerate(MCH):
                    ps = mmp.tile([128, 512], F32, tag="mm", name="mm")
                    for ki in range(4):
                        ksz = 128 if ki % 2 == 0 else 72
                        nc.tensor.matmul(
                            out=ps[:m1 - m0, :], lhsT=gw1_k[ki][:ksz, m0:m1],
                            rhs=rhs_k[ki][:ksz, :], start=(ki == 0), stop=(ki == 3))
                    if mi == 0:
                        nc.scalar.activation(out=hta[:, :], in_=ps[:128, :],
                                             func=AF.Relu, bias=gb1_t[0][:])
                    else:
                        nc.scalar.activation(out=htb[:72, :], in_=ps[:72, :],
                                             func=AF.Relu, bias=gb1_t[1][:])
                # G2 token-major + relu + mask + v-aggregation
                ps_vg = mmp.tile([128, 512], F32, tag="mm", name="psvg")
                for blk in range(4):
                    c = c0 + blk
                    ps = mmp.tile([128, 512], F32, tag="mm", name="mm")
                    nc.tensor.matmul(out=ps[:, :D], lhsT=hta[:, blk * 128:(blk + 1) * 128],
                                     rhs=gw2_k[0][:, :], start=True, stop=False)
                    nc.tensor.matmul(out=ps[:, :D], lhsT=htb[:, blk * 128:(blk + 1) * 128],
                                     rhs=gw2_k[1][:, :], start=False, stop=True)
                    g2o = gp.tile([128, D], BF16, tag="g2o", name="g2o")
                    nc.vector.tensor_scalar(
                        out=g2o[:], in0=ps[:, :D], scalar1=0.0,
                        scalar2=mask_tok[s][:, c:c + 1],
                        op0=OP.max, op1=OP.mult)
                    bex = c // 2
                    nc.tensor.matmul(
                        out=ps_vg[:BPC, :D],
                        lhsT=qsel[:, BPC - 1 - bex:2 * BPC - 1 - bex],
                        rhs=g2o[:, :], start=(blk == 0), stop=(blk == 3))
                    if debug_taps and c == 0 and s == 0:
                        t = tap("g2o0", [128, D])
                        if t is not None:
                            g2f = sp.tile([128, D], F32, tag="g2f", name="g2f")
                            nc.vector.tensor_copy(out=g2f[:], in_=g2o[:])
                            nc.sync.dma_start(out=t[:], in_=g2f[:])
                # fold this group's partial into the SBUF v accumulator
                nc.vector.tensor_tensor(
                    out=v_sb[:, s, :], in0=v_sb[:, s, :], in1=ps_vg[:BPC, :D],
                    op=OP.add)

        pend = stage_a(0)
        for g in range(NGRP):
            stage_bc(g, *pend)
            pend = stage_a(g + 1) if g + 1 < NGRP else None

        # ---------- H MLP tail ----------
        Vb = [vp.tile([BPC, D], BF16, tag=f"V{s}", name=f"V{s}") for s in range(2)]
        for s in range(2):
            nc.vector.tensor_copy(out=Vb[s][:], in_=v_sb[:, s, :])
        if debug_taps:
            t = tap("V0", [BPC, D])
            if t is not None:
                nc.sync.dma_start(out=t[:], in_=v_sb[:, 0, :])
        # transpose V -> feature-major vT chunks
        vT = []  # 4 chunks: v1a[128,16] v1b[72,16] v2a[128,16] v2b[73,16(+ones)]
        for s in range(2):
            tr = trp.tile([128, 3, 256], BF16, tag="tr", name="tr")
            for mi, (m0, m1) in enumerate(MCH):
                nc.tensor.transpose(out=tr[:m1 - m0, mi, :BPC],
                                    in_=Vb[s][:, m0:m1], identity=identb[:BPC, :BPC])
            va = vp.tile([128, BPC], BF16, tag=f"vTa{s}", name=f"vTa{s}")
            nc.vector.tensor_copy(out=va[:], in_=tr[:128, 0, :BPC])
            szb = 73 if s == 1 else 72
            vb = vp.tile([szb, BPC], BF16, tag=f"vTb{s}", name=f"vTb{s}")
            if s == 1:
                nc.vector.memset(vb[:, :], 1.0)
            nc.vector.tensor_copy(out=vb[:72, :], in_=tr[:72, 1, :BPC])
            vT += [va, vb]

        h1a = vp.tile([128, BPC], BF16, tag="h1a", name="h1a")
        h1b = vp.tile([73, BPC], BF16, tag="h1b", name="h1b")
        nc.vector.memset(h1b[:, :], 1.0)
        for mi, (m0, m1) in enumerate(MCH):
            ps = mmp.tile([128, 512], F32, tag="mm", name="mm")
            for ki in range(4):
                ksz = [128, 72, 128, 73][ki]
                nc.tensor.matmul(out=ps[:m1 - m0, :BPC], lhsT=hw1_k[ki][:ksz, m0:m1],
                                 rhs=vT[ki][:ksz, :], start=(ki == 0), stop=(ki == 3))
            if mi == 0:
                nc.scalar.activation(out=h1a[:, :], in_=ps[:128, :BPC], func=AF.Relu)
            else:
                nc.scalar.activation(out=h1b[:72, :], in_=ps[:72, :BPC], func=AF.Relu)
        h2a = vp.tile([128, BPC], BF16, tag="h2a", name="h2a")
        h2b = vp.tile([73, BPC], BF16, tag="h2b", name="h2b")
        nc.vector.memset(h2b[:, :], 1.0)
        for mi, (m0, m1) in enumerate(MCH):
            ps = mmp.tile([128, 512], F32, tag="mm", name="mm")
            for ki in range(2):
                ksz = [128, 73][ki]
                nc.tensor.matmul(out=ps[:m1 - m0, :BPC], lhsT=hw2_k[ki][:ksz, m0:m1],
                                 rhs=[h1a, h1b][ki][:ksz, :], start=(ki == 0), stop=(ki == 1))
            if mi == 0:
                nc.scalar.activation(out=h2a[:, :], in_=ps[:128, :BPC], func=AF.Relu)
            else:
                nc.scalar.activation(out=h2b[:72, :], in_=ps[:72, :BPC], func=AF.Relu)
        ps = mmp.tile([128, 512], F32, tag="mm", name="mm")
        for ki in range(2):
            ksz = [128, 73][ki]
            nc.tensor.matmul(out=ps[:C, :BPC], lhsT=wout_k[ki][:ksz, :],
                             rhs=[h2a, h2b][ki][:ksz, :], start=(ki == 0), stop=(ki == 1))
        y_sb = vp.tile([C, BPC], F32, tag="ysb", name="ysb")
        nc.vector.tensor_copy(out=y_sb[:], in_=ps[:C, :BPC])
        nc.sync.dma_start(out=y_out[:], in_=y_sb[:])

    nc.finalize()
    return nc, taps


def _host_prep(inputs):
    """Build per-core input maps from full inputs."""
    x1 = np.asarray(inputs['x1'], dtype=np.int32)
    x2 = np.asarray(inputs['x2'], dtype=np.int32)
    len1 = np.asarray(inputs['len1'], dtype=np.int64)
    len2 = np.asarray(inputs['len2'], dtype=np.int64)
    emb = np.ascontiguousarray(np.asarray(inputs['emb'], dtype=np.float32))

    def bf(x):
        return np.asarray(x, dtype=np.float32).astype(bfloat16)

    gw2e = np.vstack([np.asarray(inputs['G_W2'], np.float32),
                      np.asarray(inputs['G_b2'], np.float32)[None, :]])
    hw1e = np.vstack([np.asarray(inputs['H_W1'], np.float32),
                      np.asarray(inputs['H_b1'], np.float32)[None, :]])
    hw2e = np.vstack([np.asarray(inputs['H_W2'], np.float32),
                      np.asarray(inputs['H_b2'], np.float32)[None, :]])
    woute = np.vstack([np.asarray(inputs['W_out'], np.float32),
                       np.asarray(inputs['b_out'], np.float32)[None, :]])
    qsel = np.zeros((128, 2 * BPC - 1), dtype=np.float32)
    qsel[:, BPC - 1] = 1.0

    wsrc = {
        "identb": np.eye(128, dtype=np.float32),
        "qsel": qsel,
        "wp0": np.asarray(inputs['W_proj'], np.float32)[0:128],
        "wp1": np.asarray(inputs['W_proj'], np.float32)[128:256],
        "wp2": np.asarray(inputs['W_proj'], np.float32)[256:E],
        "fw1a": np.asarray(inputs['F_W1'], np.float32)[0:128],
        "fw1b": np.asarray(inputs['F_W1'], np.float32)[128:D],
        "fw2a": np.asarray(inputs['F_W2'], np.float32)[0:128],
        "fw2b": np.asarray(inputs['F_W2'], np.float32)[128:D],
        "gw1a": np.asarray(inputs['G_W1'], np.float32)[0:128],
        "gw1b": np.asarray(inputs['G_W1'], np.float32)[128:200],
        "gw1c": np.asarray(inputs['G_W1'], np.float32)[200:328],
        "gw1d": np.asarray(inputs['G_W1'], np.float32)[328:400],
        "gw2a": gw2e[0:128], "gw2b": gw2e[128:201],
        "hw1a": hw1e[0:128], "hw1b": hw1e[128:200],
        "hw1c": hw1e[200:328], "hw1d": hw1e[328:401],
        "hw2a": hw2e[0:128], "hw2b": hw2e[128:201],
        "woa": woute[0:128], "wob": woute[128:201],
    }
    wpack = np.zeros((128, WCOLS), dtype=bfloat16)
    for nm, (c0, rows, ncol) in WOFF.items():
        wpack[:rows, c0:c0 + ncol] = bf(wsrc[nm])
    wpack = np.ascontiguousarray(wpack)

    fb1 = np.asarray(inputs['F_b1'], np.float32)
    fb2 = np.asarray(inputs['F_b2'], np.float32)
    gb1 = np.asarray(inputs['G_b1'], np.float32)

    iota = np.arange(128, dtype=np.float32)

    in_maps = []
    for core in range(NCORES):
        sl = slice(core * BPC, (core + 1) * BPC)

        def idx_of(x):
            return np.ascontiguousarray(
                x[sl].reshape(-1).reshape(NB, 128).T.astype(np.int32))

        def mask_of(ln):
            c = np.arange(NB)
            thr = ln[sl][c // 2] - 128.0 * (c % 2)
            return (iota[:, None] < thr[None, :]).astype(np.float32)

        fpack = np.zeros((128, FCOLS), dtype=np.float32)
        fpack[:, 0:NB] = mask_of(len1)
        fpack[:, NB:2 * NB] = mask_of(len2)
        fpack[0:128, FB_COLS["fb1a"]] = fb1[0:128]
        fpack[0:72, FB_COLS["fb1b"]] = fb1[128:D]
        fpack[0:128, FB_COLS["fb2a"]] = fb2[0:128]
        fpack[0:72, FB_COLS["fb2b"]] = fb2[128:D]
        fpack[0:128, FB_COLS["gb1a"]] = gb1[0:128]
        fpack[0:72, FB_COLS["gb1b"]] = gb1[128:D]

        in_maps.append(dict(
            emb=emb, x1i=idx_of(x1), x2i=idx_of(x2),
            wpack=wpack, fpack=np.ascontiguousarray(fpack),
        ))
    return in_maps


def run(inputs, debug_taps=False, trace=False):
    key = debug_taps
    if key not in _cache:
        _cache[key] = _build(debug_taps)
    nc, taps = _cache[key]
    in_maps = _host_prep(inputs)
    from concourse.bass_utils import run_bass_kernel_spmd
    res = run_bass_kernel_spmd(nc, in_maps, list(range(NCORES)), trace=trace)
    y = np.concatenate([r['y'].T for r in res.results], axis=0)
    return y.astype(np.float32), res


def kernel(**inputs) -> np.ndarray:
    y, _ = run(inputs)
    return y
